# revision 36
# baseline (speedup 1.0000x reference)
"""Trainium2 Bass kernel for DepthwiseSeparableConv3d (inference).

Problem: x[2,48,48,48,64] -> dw3x3x3 depthwise + BN + ReLU -> 1x1x1 conv
(64->128) + BN + ReLU -> z[2,48,48,48,128], all f32.

Strategy (8 NeuronCores, data-parallel over (b, h-quarter) slabs):
 - Each core owns batch b = core//4 and h-rows [12*hq, 12*hq+12) for
   hq = core%4, full D and W, with SAME-pad halos baked in on host.
 - Depthwise conv runs on TensorE as a 2D block-Toeplitz matmul:
   K = 120 partitions = (2 channels x 6x10 (h,w) input patch),
   M = 64 partitions  = (2 channels x 4x8 (h,w) output patch).
   The 9 (dy,dx) taps live in the Toeplitz weight; the 3 dz taps are
   PSUM-accumulated matmuls against d-shifted views of the same SBUF
   tile.  TWO channel-pairs share each PSUM tile via PE column tiling
   (tile_position cols 0/64), so BN1 and the regroup DMA run once per
   4 channels.  One weight load serves both d-half matmuls.
 - DMA layouts are chosen for >=1.7KB contiguous descriptor runs:
   x is k-major (7.2KB runs), z is written as contiguous [F, 4*512]
   DRAM blocks (4KB runs), the regroup moves full-depth rows (1.7KB).
 - BN scales are folded into the conv weights so BN+ReLU is a single
   biased-ReLU activation (BN1, ScalarE) or add+max tensor_scalar
   (BN2, split DVE/ScalarE).
 - A per-quad SBUF->SBUF DMA (alternating Pool SWDGE / ACT HWDGE
   queues) regroups (pp,c2,ho,wo)-partitions into channel partitions.
 - Output z stays blocked on device; host reassembles NDHWC.
"""

import sys

for _p in ("/opt/trn_rl_repo", "/opt/pypackages"):
    if _p not in sys.path:
        sys.path.insert(0, _p)

import numpy as np
import ml_dtypes

import concourse.bass as bass
import concourse.tile as tile
from concourse import bacc, mybir
from concourse.bass_utils import run_bass_kernel_spmd

# ----- problem constants (hardcoded per spec) -----
B, D, H, W, C, F = 2, 48, 48, 48, 64, 128
EPS = 1e-3
N_CORES = 8
HQ = H // 4                       # 12 h-rows per core
NP = C // 2                       # 32 channel-pairs
NQ = NP // 2                      # 16 quads (2 pairs each)
POH, POW = 4, 8                   # patch out edges (ho, wo)
PIH, PIW = POH + 2, POW + 2       # 6, 10
TH = HQ // POH                    # 3 h-tiles
TW = W // POW                     # 6 w-tiles
MP = 64                           # output partitions per pair
KP = 2 * PIH * PIW                # 120 input partitions
NB_ = POH * POW                   # 32 (ho,wo) blocks
DI = D + 2                        # 50 padded d slices
NTT = TH * TW                     # 18 (th,tw) tiles
DH = D // 2                       # 24 outputs per d-half matmul
NA = DH * NTT                     # 432 moving cols per (pair,dz,half)
NF = 2 * NA                       # 864 positions per (quad-channel, block)
GRP = 8                           # pairs per x tile
NG = NP // GRP                    # 4 groups
ZC = 512                          # pointwise chunk (PSUM cols, 1 bank)
NCH = NB_ * NF // ZC              # 54 pw chunks
ZB = 4                            # chunks per z DRAM block
NZB = (NCH + ZB - 1) // ZB        # 14 z blocks
NPOS = NB_ * NF                   # 27648 positions per core

BF16 = mybir.dt.bfloat16
F32 = mybir.dt.float32
RELU = mybir.ActivationFunctionType.Relu
ADD = mybir.AluOpType.add
MAX = mybir.AluOpType.max

_COMPILED = None


def _build_bass():
    nc = bacc.Bacc("TRN2", target_bir_lowering=False, debug=False,
                   num_devices=N_CORES)

    xt_d = nc.dram_tensor("xt", [NG, KP, GRP, DI, NTT], BF16,
                          kind="ExternalInput").ap()
    wt_d = nc.dram_tensor("wt", [KP, NP, 3, MP], BF16,
                          kind="ExternalInput").ap()
    pw_d = nc.dram_tensor("pwk", [C, F], BF16, kind="ExternalInput").ap()
    c1_d = nc.dram_tensor("c1b", [2 * MP, NQ], F32,
                          kind="ExternalInput").ap()
    c2_d = nc.dram_tensor("c2b", [F, 1], F32, kind="ExternalInput").ap()
    z_d = nc.dram_tensor("z", [NZB, F, ZB * ZC], BF16,
                         kind="ExternalOutput").ap()

    with tile.TileContext(nc) as tc:
        with (
            tc.tile_pool(name="consts", bufs=1) as consts,
            tc.tile_pool(name="xt", bufs=NG) as xt_pool,
            tc.tile_pool(name="Y", bufs=1) as Y_pool,
            tc.tile_pool(name="yg", bufs=16) as yg_pool,
            tc.tile_pool(name="zbuf", bufs=4) as z_pool,
        ):
            pw_sb = consts.tile([C, F], BF16)
            c1_sb = consts.tile([2 * MP, NQ], F32)
            c2_sb = consts.tile([F, 1], F32)
            wt_sb = consts.tile([KP, NP, 3, MP], BF16)

            xg = [xt_pool.tile([KP, GRP, DI, NTT], BF16, tag="xg",
                               name=f"xg_{g}")
                  for g in range(NG)]

            # input DMAs all on the ACT ring (k-major: 7.2KB runs per
            # descriptor); the SP ring is reserved for regroup + z so
            # those never queue behind bulk loads.  Only group 0 and
            # the weights load up-front; later groups are issued
            # between quads so their configs don't head-of-line block
            # the BN1 activations.
            nc.scalar.dma_start(wt_sb[:, 0:4], wt_d[:, 0:4])
            nc.sync.dma_start(xg[0][:, 0:4], xt_d[0, :, 0:4])
            nc.scalar.dma_start(c1_sb[:], c1_d[:])
            nc.scalar.dma_start(c2_sb[:], c2_d[:])
            nc.scalar.dma_start(pw_sb[:], pw_d[:])
            nc.scalar.dma_start(wt_sb[:, 4:8], wt_d[:, 4:8])
            nc.sync.dma_start(xg[0][:, 4:GRP], xt_d[0, :, 4:GRP])

            # depthwise output, channel-partition layout
            Yt = Y_pool.tile([C, NB_, NF], BF16, name="Yt")

            rg_fifo = []

            def emit_rg(j, ygq):
                nc.gpsimd.dma_start(Yt[4 * j: 4 * j + 4], ygq[:])

            with tc.tile_pool(name="psdw", bufs=4, space="PSUM") as ps_pool:
                for j in range(NQ):
                    g, jg = j // 4, j % 4
                    psq = ps_pool.tile([2 * MP, 2, ZC], F32, tag="ps",
                                       name=f"ps_{j}")
                    for s in range(2):
                        p = 2 * j + s
                        for dz in range(3):
                            for h in range(2):
                                rhs = xg[g][:, 2 * jg + s,
                                            h * DH + dz:h * DH + dz + DH]
                                nc.tensor.matmul(
                                    psq[s * MP:(s + 1) * MP, h, 0:NA],
                                    wt_sb[:, p, dz], rhs,
                                    start=(dz == 0), stop=(dz == 2))
                    ygq = yg_pool.tile([2 * MP, 2, NA], BF16, tag="yg",
                                       name=f"yg_{j}")
                    nc.scalar.activation(
                        ygq[:], psq[:, :, 0:NA], RELU,
                        bias=c1_sb[:, j:j + 1])
                    # regroup (pp,c2,ho,wo)->channel partitions on the
                    # Pool SWDGE queue (cheap desc gen, spreads over
                    # all 16 engines); lag 4 quads behind DW so the
                    # SBUF->SBUF traffic stays off the x read stream
                    rg_fifo.append((j, ygq))
                    if len(rg_fifo) > 8:
                        emit_rg(*rg_fifo.pop(0))
                    # trickle the next x group load between quads,
                    # alternating SP/ACT queues
                    if j in (1, 3, 5):
                        g2 = (j + 1) // 2
                        eng = nc.sync if g2 == 2 else nc.scalar
                        eng.dma_start(xg[g2][:, 0:4], xt_d[g2, :, 0:4])
                        eng.dma_start(xg[g2][:, 4:GRP],
                                      xt_d[g2, :, 4:GRP])
                        w0 = 8 * g2
                        nc.scalar.dma_start(wt_sb[:, w0:w0 + 8],
                                            wt_d[:, w0:w0 + 8])

            while rg_fifo:
                emit_rg(*rg_fifo.pop(0))

            with tc.tile_pool(name="pspw", bufs=6, space="PSUM") as pw_pool:
                zq = {"n": 0, "blk": 0}

                def flush_z():
                    k = zq["n"]
                    if not k:
                        return
                    nc.gpsimd.dma_start(
                        z_d[zq["blk"], :, 0:k * ZC],
                        zq["t"][:, 0:k].rearrange("f s r -> f (s r)"))
                    zq["n"] = 0
                    zq["blk"] += 1

                for q in range(NCH):
                    Yv = Yt[:].rearrange("c a b -> c (a b)")
                    pps = pw_pool.tile([F, ZC], F32, tag="pwps",
                                       name=f"pps_{q}")
                    nc.tensor.matmul(pps[:], pw_sb[:],
                                     Yv[:, q * ZC:(q + 1) * ZC],
                                     start=True, stop=True)
                    if zq["n"] == 0:
                        zq["t"] = z_pool.tile([F, ZB, ZC], BF16, tag="zt",
                                              name=f"zt_{q}")
                    s = zq["n"]
                    zt = zq["t"]
                    if q % 2 == 0:
                        nc.scalar.activation(zt[:, s], pps[:], RELU,
                                             bias=c2_sb[:, 0:1])
                    else:
                        nc.vector.tensor_scalar(zt[:, s], pps[:],
                                                c2_sb[:, 0:1], 0.0,
                                                ADD, MAX)
                    zq["n"] = s + 1
                    if zq["n"] == ZB:
                        flush_z()
                flush_z()

    nc.compile()
    return nc


def _prep_inputs(x, dw_kernel, dw_bias, bn1_gamma, bn1_beta, bn1_mean,
                 bn1_var, pw_kernel, pw_bias, bn2_gamma, bn2_beta, bn2_mean,
                 bn2_var):
    """Build per-core input maps (numpy only, off the device clock)."""
    x = np.asarray(x, np.float32)
    dw = np.asarray(dw_kernel, np.float32)[:, :, :, 0, :]     # [3,3,3,C]
    a1 = np.asarray(bn1_gamma, np.float32) / np.sqrt(
        np.asarray(bn1_var, np.float32) + EPS)
    c1 = a1 * (np.asarray(dw_bias, np.float32)
               - np.asarray(bn1_mean, np.float32)) \
        + np.asarray(bn1_beta, np.float32)
    a2 = np.asarray(bn2_gamma, np.float32) / np.sqrt(
        np.asarray(bn2_var, np.float32) + EPS)
    c2 = a2 * (np.asarray(pw_bias, np.float32)
               - np.asarray(bn2_mean, np.float32)) \
        + np.asarray(bn2_beta, np.float32)

    # Toeplitz weights [KP, NP, 3, MP], a1 prefolded, k-major
    aw = dw * a1[None, None, None, :]                         # [3,3,3,C]
    wt = np.zeros((2, PIH, PIW, NP, 3, 2, POH, POW), np.float32)
    c2i = np.arange(2)[:, None, None]
    hoi = np.arange(POH)[None, :, None]
    woi = np.arange(POW)[None, None, :]
    for dy in range(3):
        for dx in range(3):
            # value for [c2, ho, wo, p, dz] = aw[dz, dy, dx, 2p+c2]
            val = aw[:, dy, dx, :].reshape(3, NP, 2)          # [dz, p, c2]
            val = val.transpose(2, 1, 0)[:, None, None]       # [2,1,1,NP,3]
            wt[c2i, dy + hoi, dx + woi, :, :, c2i, hoi, woi] = val
    wt = wt.reshape(KP, NP, 3, MP).astype(ml_dtypes.bfloat16)

    # c1b[(pp,c2,ho,wo), j] = c1[4j + 2pp + c2]
    c1b = np.ascontiguousarray(
        np.repeat(c1.reshape(NQ, 4).T, NB_, axis=0).reshape(2 * MP, NQ)
    ).astype(np.float32)
    pwk = (np.asarray(pw_kernel, np.float32)
           * a2[None, :]).astype(ml_dtypes.bfloat16)
    c2b = c2.reshape(F, 1).astype(np.float32)

    # x padded once globally: [B, D+2, H+2, W+2, C]
    xp = np.zeros((B, D + 2, H + 2, W + 2, C), np.float32)
    xp[:, 1:-1, 1:-1, 1:-1, :] = x

    widx = (np.arange(TW)[:, None] * POW + np.arange(PIW)[None, :])
    hidx = (np.arange(TH)[:, None] * POH + np.arange(PIH)[None, :])
    in_maps = []
    for core in range(N_CORES):
        b, hq = core // 4, core % 4
        slab = xp[b, :, hq * HQ: hq * HQ + HQ + 2]        # [50, 14, 50, C]
        t = slab[:, :, widx.ravel()].reshape(DI, HQ + 2, TW, PIW, C)
        t = t[:, hidx.ravel()].reshape(DI, TH, PIH, TW, PIW, C)
        # [d, th, hy, tw, wx, c] -> [c, hy, wx, d, th, tw]
        t = t.transpose(5, 2, 4, 0, 1, 3)
        xt = t.reshape(NP, KP, DI * NTT)
        # k-major groups: [NG, KP, GRP, DI, NTT]
        xt = xt.reshape(NG, GRP, KP, DI * NTT).transpose(0, 2, 1, 3)
        xt = xt.reshape(NG, KP, GRP, DI, NTT)
        in_maps.append({
            "xt": np.ascontiguousarray(xt).astype(ml_dtypes.bfloat16),
            "wt": wt, "pwk": pwk, "c1b": c1b, "c2b": c2b,
        })
    return in_maps


def _gather_output(results):
    z = np.empty((B, D, H, W, F), np.float32)
    for core in range(N_CORES):
        b, hq = core // 4, core % 4
        zc = np.asarray(results[core]["z"], dtype=np.float32)
        zc = zc.reshape(NZB, F, ZB, ZC).transpose(1, 0, 2, 3) \
               .reshape(F, NZB * ZB * ZC)[:, 0:NPOS]
        # cols = (ho, wo, dhalf, d', th, tw)
        v = zc.reshape(F, POH, POW, 2, DH, TH, TW)
        # -> [dhalf, d', th, ho, tw, wo, f]
        v = v.transpose(3, 4, 5, 1, 6, 2, 0)
        z[b, :, hq * HQ: hq * HQ + HQ] = v.reshape(D, HQ, W, F)
    return z


def kernel(**inputs):
    global _COMPILED
    if _COMPILED is None:
        _COMPILED = _build_bass()
    in_maps = _prep_inputs(**inputs)
    res = run_bass_kernel_spmd(_COMPILED, in_maps,
                               core_ids=list(range(N_CORES)))
    return _gather_output(res.results)


if __name__ == "__main__":
    pass
